# revision 1
# baseline (speedup 1.0000x reference)
"""Multi-head dot-product attention on 8 TRN2 NeuronCores.

Problem: B=4, S=2048, D=1024, H=16, DH=64 (fp32 reference).

Sharding: 8 shards = 4 batches x 2 head-halves. Each core computes, for one
batch b and 8 heads, the QKV projections, attention, and its partial output
projection. The host sums the two half-head partials per batch (the Wo
contraction all-reduce) and adds bo.

Per-core kernel layout (all matmul contraction dims on SBUF partitions):
  - XqT/XkvT: x loaded transposed, [D(128-tiles), S] fp32; projections run in
    float32r (full-rate PE) so no input casts are needed.
  - QT/KT: [128 = head-pair (2x64 dh), S] fp16 - produced directly transposed
    by using W as lhsT. Head pairs stacked so that the K=64 scores matmuls for
    the two heads row-pack onto the PE array (tile_position rows 0-63/64-127).
  - scoresT: [k-tile 128, q 1024] PSUM; exp on ACT (scale=1/8 folded in,
    no max-subtraction: scores ~ N(0,1), |s| < ~7, exp is safe in fp32/fp16).
  - softmax denominators: DVE accumulates expT k-tiles into an fp16 partial-sum
    tile; a ones[128,64] matmul reduces partitions AND broadcasts, giving
    per-head denominator rows aligned with xT; fast reciprocal on DVE.
  - PV: xT[dh, q] accumulated over k-tiles, two heads col-packed
    (tile_position cols 0-63/64-127) -> xT stacked [128, q] ready as lhsT for
    the Wo projection. Normalization fused into the PSUM->SBUF evacuation.
  - out projection: out[q,d] accumulated over 4 head-pairs, DMA'd to DRAM
    straight from PSUM.
"""

import os

import numpy as np

import concourse.bass as bass
from concourse import bacc
import concourse.mybir as mybir
import concourse.tile as tile
from concourse.bass_utils import run_bass_kernel_spmd

B, S, D, H, DH = 4, 2048, 1024, 16, 64
P = 128
HC = H // 2          # heads per core = 8
PAIRS = HC // 2      # head pairs per core = 4
DT = D // P          # projection contraction tiles = 8
NKT = S // P         # key tiles = 16
QC = 1024            # q chunk per psum tile
NQC = S // QC        # 2
NSUB = QC // 512     # matmul sub-chunks per psum tile
HDH = HC * DH        # per-core Wo contraction = 512

F32 = mybir.dt.float32
F16 = mybir.dt.float16
EXP = mybir.ActivationFunctionType.Exp


def _emit(nc):
    xq = nc.dram_tensor("xq", [S, D], F16, kind="ExternalInput")
    xkv = nc.dram_tensor("xkv", [S, D], F16, kind="ExternalInput")
    wq = nc.dram_tensor("wq", [D, HDH], F16, kind="ExternalInput")
    wk = nc.dram_tensor("wk", [D, HDH], F16, kind="ExternalInput")
    wv = nc.dram_tensor("wv", [D, HDH], F16, kind="ExternalInput")
    bq = nc.dram_tensor("bq", [HDH], F16, kind="ExternalInput")
    bk = nc.dram_tensor("bk", [HDH], F16, kind="ExternalInput")
    bv = nc.dram_tensor("bv", [HDH], F16, kind="ExternalInput")
    wo = nc.dram_tensor("wo", [HDH, D], F16, kind="ExternalInput")
    out = nc.dram_tensor("out", [S, D], F32, kind="ExternalOutput")

    with tile.TileContext(nc) as tc:
        with tc.tile_pool(name="persist", bufs=1) as pers:
            # persistent SBUF tensors
            qt_sb = [pers.tile([P, S], F16, tag=f"qt{t}", name=f"qt{t}") for t in range(PAIRS)]
            kt_sb = [pers.tile([P, S], F16, tag=f"kt{t}", name=f"kt{t}") for t in range(PAIRS)]
            v_sb = [pers.tile([P, HDH], F16, tag=f"v{st}", name=f"v{st}") for st in range(NKT)]
            wo_sb = [pers.tile([P, D], F16, tag=f"wo{t}", name=f"wo{t}") for t in range(PAIRS)]
            ones_mm = pers.tile([1, 512], F16, tag="ones_mm")
            ones_red = pers.tile([P, 64], F16, tag="ones_red")
            bq_sb = pers.tile([1, HDH], F16, tag="bq")
            bk_sb = pers.tile([1, HDH], F16, tag="bk")
            bv_sb = pers.tile([1, HDH], F16, tag="bv")

            nc.vector.memset(ones_mm, 1.0)
            nc.vector.memset(ones_red, 1.0)
            nc.sync.dma_start(out=bq_sb, in_=bq[None, :])
            nc.sync.dma_start(out=bk_sb, in_=bk[None, :])
            nc.sync.dma_start(out=bv_sb, in_=bv[None, :])

            # ---------------- Phase 1: projections ----------------
            with (
                tc.tile_pool(name="xt", bufs=9) as xt_pool,
                tc.tile_pool(name="w", bufs=16) as w_pool,
                tc.tile_pool(name="pproj", bufs=4, space="PSUM") as pj,
            ):
                # Wo load
                for t in range(PAIRS):
                    nc.sync.dma_start(out=wo_sb[t], in_=wo[t * P : (t + 1) * P, :])

                def load_xT(x_dram):
                    # One big M2S XBAR transpose per d-tile (fp16, DRAM->SBUF)
                    tiles = []
                    for d in range(DT):
                        t_ = xt_pool.tile([P, S], F16, tag="xt")
                        nc.sync.dma_start_transpose(
                            out=t_, in_=x_dram[:, d * P : (d + 1) * P]
                        )
                        tiles.append(t_)
                    return tiles

                def load_w(w_dram):
                    tiles = []
                    for d in range(DT):
                        t_ = w_pool.tile([P, HDH], F16, tag="w")
                        nc.sync.dma_start(out=t_, in_=w_dram[d * P : (d + 1) * P, :])
                        tiles.append(t_)
                    return tiles

                def proj_T(x_tiles, w_tiles, b_sb, out_tiles):
                    # out_tiles[pair][128 = pair-dh, S] = W.T @ X.T + b
                    for t in range(PAIRS):
                        for c in range(S // 512):
                            ps = pj.tile([P, 512], F32, tag="pj")
                            for d in range(DT):
                                nc.tensor.matmul(
                                    ps,
                                    lhsT=w_tiles[d][:, t * P : (t + 1) * P],
                                    rhs=x_tiles[d][:, c * 512 : (c + 1) * 512],
                                    start=(d == 0),
                                    stop=False,
                                )
                            nc.tensor.matmul(
                                ps,
                                lhsT=b_sb[:, t * P : (t + 1) * P],
                                rhs=ones_mm,
                                start=False,
                                stop=True,
                            )
                            nc.vector.tensor_copy(
                                out=out_tiles[t][:, c * 512 : (c + 1) * 512], in_=ps
                            )

                xq_t = load_xT(xq)
                wq_t = load_w(wq)
                wk_t = load_w(wk)
                proj_T(xq_t, wq_t, bq_sb, qt_sb)

                xkv_t = load_xT(xkv)
                proj_T(xkv_t, wk_t, bk_sb, kt_sb)

                wv_t = load_w(wv)
                # V natural layout: [s-tile 128, (h dh) 512] = X @ Wv + bv
                for st in range(NKT):
                    ps = pj.tile([P, 512], F32, tag="pj")
                    for d in range(DT):
                        nc.tensor.matmul(
                            ps,
                            lhsT=xkv_t[d][:, st * P : (st + 1) * P],
                            rhs=wv_t[d],
                            start=(d == 0),
                            stop=False,
                        )
                    nc.tensor.matmul(
                        ps,
                        lhsT=ones_mm[:, :P],
                        rhs=bv_sb,
                        start=False,
                        stop=True,
                    )
                    nc.vector.tensor_copy(out=v_sb[st], in_=ps)

            # ---------------- Phase 2: attention + out projection ----------------
            with (
                tc.tile_pool(name="psc", bufs=3, space="PSUM") as psc,
                tc.tile_pool(name="pxt", bufs=1, space="PSUM") as pxt,
                tc.tile_pool(name="et", bufs=4) as et_pool,
                tc.tile_pool(name="accp", bufs=4) as acc_pool,
                tc.tile_pool(name="rec", bufs=2) as rec_pool,
                tc.tile_pool(name="xtsb", bufs=8) as xtsb_pool,
            ):
                for qc in range(NQC):
                    xts = []
                    for pr in range(PAIRS):
                        h0, h1 = 2 * pr, 2 * pr + 1
                        acc0 = acc_pool.tile([P, QC], F16, tag="acc")
                        acc1 = acc_pool.tile([P, QC], F16, tag="acc")
                        xt_ps = pxt.tile([P, QC], F32, tag="xt")
                        for kt in range(NKT):
                            ps_a = psc.tile([P, QC], F32, tag="sc")
                            ps_b = psc.tile([P, QC], F32, tag="sc")
                            ksl = slice(kt * P, (kt + 1) * P)
                            for sub in range(NSUB):
                                sl = slice(sub * 512, (sub + 1) * 512)
                                qsl = slice(
                                    qc * QC + sub * 512, qc * QC + (sub + 1) * 512
                                )
                                # scoresT[k, q] for the two heads, row-packed
                                nc.tensor.matmul(
                                    ps_a[:, sl],
                                    lhsT=kt_sb[pr][0:64, ksl],
                                    rhs=qt_sb[pr][0:64, qsl],
                                    start=True,
                                    stop=True,
                                    tile_position=(0, 0),
                                )
                                nc.tensor.matmul(
                                    ps_b[:, sl],
                                    lhsT=kt_sb[pr][64:128, ksl],
                                    rhs=qt_sb[pr][64:128, qsl],
                                    start=True,
                                    stop=True,
                                    tile_position=(64, 0),
                                )
                            et0 = et_pool.tile([P, QC], F16, tag="et")
                            et1 = et_pool.tile([P, QC], F16, tag="et")
                            nc.scalar.activation(out=et0, in_=ps_a, func=EXP, scale=0.125)
                            nc.scalar.activation(out=et1, in_=ps_b, func=EXP, scale=0.125)
                            # partial softmax denominators (fp16, 16 terms per lane)
                            if kt == 0:
                                nc.vector.tensor_copy(out=acc0, in_=et0)
                                nc.vector.tensor_copy(out=acc1, in_=et1)
                            else:
                                nc.vector.tensor_add(out=acc0, in0=acc0, in1=et0)
                                nc.vector.tensor_add(out=acc1, in0=acc1, in1=et1)
                            # xT[dh, q] += V[k-tile].T-slice @ expT, heads col-packed
                            for sub in range(NSUB):
                                sl = slice(sub * 512, (sub + 1) * 512)
                                nc.tensor.matmul(
                                    xt_ps[0:64, sl],
                                    lhsT=v_sb[kt][:, h0 * DH : (h0 + 1) * DH],
                                    rhs=et0[:, sl],
                                    start=(kt == 0),
                                    stop=(kt == NKT - 1),
                                    tile_position=(0, 0),
                                    skip_group_check=True,
                                )
                                nc.tensor.matmul(
                                    xt_ps[64:128, sl],
                                    lhsT=v_sb[kt][:, h1 * DH : (h1 + 1) * DH],
                                    rhs=et1[:, sl],
                                    start=(kt == 0),
                                    stop=(kt == NKT - 1),
                                    tile_position=(0, 64),
                                    skip_group_check=True,
                                )
                        # denominators: partition-reduce + broadcast in one matmul
                        bs = psc.tile([P, QC], F32, tag="sc")
                        for sub in range(NSUB):
                            sl = slice(sub * 512, (sub + 1) * 512)
                            nc.tensor.matmul(
                                bs[0:64, sl],
                                lhsT=ones_red,
                                rhs=acc0[:, sl],
                                start=True,
                                stop=True,
                                tile_position=(0, 0),
                                skip_group_check=True,
                            )
                            nc.tensor.matmul(
                                bs[64:128, sl],
                                lhsT=ones_red,
                                rhs=acc1[:, sl],
                                start=True,
                                stop=True,
                                tile_position=(0, 64),
                                skip_group_check=True,
                            )
                        rec = rec_pool.tile([P, QC], F32, tag="rec")
                        nc.vector.reciprocal_approx_fast(out=rec, in_=bs)
                        xt_sb = xtsb_pool.tile([P, QC], F16, tag="xtsb")
                        nc.vector.tensor_mul(out=xt_sb, in0=xt_ps, in1=rec)
                        xts.append(xt_sb)
                    # out projection for this q chunk
                    for qt_ in range(QC // P):
                        for dc in range(D // 512):
                            po = psc.tile([P, 512], F32, tag="sc")
                            for pr in range(PAIRS):
                                nc.tensor.matmul(
                                    po,
                                    lhsT=xts[pr][:, qt_ * P : (qt_ + 1) * P],
                                    rhs=wo_sb[pr][:, dc * 512 : (dc + 1) * 512],
                                    start=(pr == 0),
                                    stop=(pr == PAIRS - 1),
                                )
                            osb = xtsb_pool.tile([P, 512], F32, tag="osb")
                            nc.vector.tensor_copy(out=osb, in_=po)
                            q0 = qc * QC + qt_ * P
                            nc.gpsimd.dma_start(
                                out=out[q0 : q0 + P, dc * 512 : (dc + 1) * 512],
                                in_=osb,
                            )
    return nc


_NC_CACHE = None
LAST_RESULTS = None


def _get_nc():
    global _NC_CACHE
    if _NC_CACHE is None:
        nc = bacc.Bacc(None, target_bir_lowering=False)
        _emit(nc)
        nc.compile()
        _NC_CACHE = nc
    return _NC_CACHE


def kernel(**inputs):
    global LAST_RESULTS
    inputs_q = np.ascontiguousarray(inputs["inputs_q"], np.float16)
    inputs_kv = np.ascontiguousarray(inputs["inputs_kv"], np.float16)
    Wq = np.asarray(inputs["Wq"], np.float16)
    Wk = np.asarray(inputs["Wk"], np.float16)
    Wv = np.asarray(inputs["Wv"], np.float16)
    bq = np.asarray(inputs["bq"], np.float16)
    bk = np.asarray(inputs["bk"], np.float16)
    bv = np.asarray(inputs["bv"], np.float16)
    Wo = np.asarray(inputs["Wo"], np.float16)
    bo = np.asarray(inputs["bo"], np.float32)

    nc = _get_nc()

    in_maps = []
    for core in range(8):
        b, g = core // 2, core % 2
        hsl = slice(g * HC, (g + 1) * HC)
        in_maps.append(
            {
                "xq": inputs_q[b],
                "xkv": inputs_kv[b],
                "wq": np.ascontiguousarray(Wq[:, hsl, :].reshape(D, HDH)),
                "wk": np.ascontiguousarray(Wk[:, hsl, :].reshape(D, HDH)),
                "wv": np.ascontiguousarray(Wv[:, hsl, :].reshape(D, HDH)),
                "bq": np.ascontiguousarray(bq[hsl].reshape(HDH)),
                "bk": np.ascontiguousarray(bk[hsl].reshape(HDH)),
                "bv": np.ascontiguousarray(bv[hsl].reshape(HDH)),
                "wo": np.ascontiguousarray(Wo[hsl].reshape(HDH, D)),
            }
        )

    res = run_bass_kernel_spmd(
        nc,
        in_maps,
        core_ids=list(range(8)),
        trace=bool(int(os.environ.get("KERNEL_TRACE", "0"))),
    )
    LAST_RESULTS = res

    out = np.empty((B, S, D), np.float32)
    for b in range(B):
        out[b] = res.results[2 * b]["out"] + res.results[2 * b + 1]["out"] + bo
    return out



# revision 3
# speedup vs baseline: 1.1897x; 1.1897x over previous
"""Multi-head dot-product attention on 8 TRN2 NeuronCores.

Problem: B=4, S=2048, D=1024, H=16, DH=64 (fp32 reference).

Sharding: 8 shards = 4 batches x 2 head-halves. Each core computes, for one
batch b and 8 heads, the QKV projections, attention, and its partial output
projection. The host sums the two half-head partials per batch (the Wo
contraction all-reduce) and adds bo.

v2: the kernel is structured around the Scalar engine (ACT), which is the
critical resource: softmax needs exp of 8*2048*2048 = 33.5M elements per core
at 128 lanes @ 1.2 GHz ~= 255us when streamed back-to-back as [128,1024]
instructions. Everything else (PE matmuls ~240us, DVE ~210us) is scheduled to
hide underneath that stream:

  - attention is blocked as (head-pair pr, q-chunk qc=512, k-tile kt=128);
    per kt: one row-packed scores matmul pair (concurrent on the PE), ONE
    [128,1024] exp covering both heads, one DVE accumulate for the softmax
    denominator, and a col-packed PV matmul pair accumulating xT in PSUM.
  - the PE stream is software-pipelined: scores(kt+2) is emitted BEFORE
    pv(kt) so the exp stream never stalls behind the PE queue; the next
    block's first two scores are emitted before the current block's epilogue.
  - blocks are ordered pr-major; projections for pair pr+1 and the output
    projection run as "fillers" in the stream's PE slack (one ~1us filler
    per two kt windows).
  - PSUM budget (8 banks): scores 2x[128,1024]f32 (4) + xT [128,512]f32 (1)
    + denominator bs [128,512] (1) + 2 filler/out-proj slots (2).
"""

import os

import numpy as np

import concourse.bass as bass
from concourse import bacc
import concourse.mybir as mybir
import concourse.tile as tile
from concourse.bass_utils import run_bass_kernel_spmd

B, S, D, H, DH = 4, 2048, 1024, 16, 64
P = 128
HC = H // 2          # heads per core = 8
PAIRS = HC // 2      # head pairs per core = 4
DT = D // P          # projection contraction tiles = 8
NKT = S // P         # key tiles = 16
QC = 512             # q chunk (per attention block)
NQC = S // QC        # 4
HDH = HC * DH        # per-core Wo contraction = 512

F32 = mybir.dt.float32
F16 = mybir.dt.float16
EXP = mybir.ActivationFunctionType.Exp


def _emit(nc):
    xq = nc.dram_tensor("xq", [S, D], F16, kind="ExternalInput")
    xkv = nc.dram_tensor("xkv", [S, D], F16, kind="ExternalInput")
    wq = nc.dram_tensor("wq", [D, HDH], F16, kind="ExternalInput")
    wk = nc.dram_tensor("wk", [D, HDH], F16, kind="ExternalInput")
    wv = nc.dram_tensor("wv", [D, HDH], F16, kind="ExternalInput")
    bq = nc.dram_tensor("bq", [HDH], F16, kind="ExternalInput")
    bk = nc.dram_tensor("bk", [HDH], F16, kind="ExternalInput")
    bv = nc.dram_tensor("bv", [HDH], F16, kind="ExternalInput")
    wo = nc.dram_tensor("wo", [HDH, D], F16, kind="ExternalInput")
    out = nc.dram_tensor("out", [S, D], F32, kind="ExternalOutput")

    with tile.TileContext(nc) as tc:
        with (
            tc.tile_pool(name="persist", bufs=1) as pers,
            tc.tile_pool(name="etp", bufs=4) as et_pool,
            tc.tile_pool(name="accp", bufs=2) as acc_pool,
            tc.tile_pool(name="recp", bufs=2) as rec_pool,
            tc.tile_pool(name="xtsb", bufs=16) as xtsb_pool,
            tc.tile_pool(name="osbp", bufs=4) as osb_pool,
            tc.tile_pool(name="psc", bufs=2, space="PSUM") as psc,
            tc.tile_pool(name="pxt", bufs=1, space="PSUM") as pxt,
            tc.tile_pool(name="pbs", bufs=1, space="PSUM") as pbs,
            tc.tile_pool(name="pfil", bufs=2, space="PSUM") as pfil,
        ):
            # ---------------- persistent SBUF ----------------
            qt_sb = [pers.tile([P, S], F16, tag=f"qt{t}", name=f"qt{t}") for t in range(PAIRS)]
            kt_sb = [pers.tile([P, S], F16, tag=f"kt{t}", name=f"kt{t}") for t in range(PAIRS)]
            v_sb = [pers.tile([P, HDH], F16, tag=f"v{st}", name=f"v{st}") for st in range(NKT)]
            wo_sb = [pers.tile([P, D], F16, tag=f"wo{t}", name=f"wo{t}") for t in range(PAIRS)]
            xkv_t = [pers.tile([P, S], F16, tag=f"xkv{d}", name=f"xkv{d}") for d in range(DT)]
            xq_t = [pers.tile([P, S], F16, tag=f"xq{d}", name=f"xq{d}") for d in range(DT)]
            wk_t = [pers.tile([P, HDH], F16, tag=f"wk{d}", name=f"wk{d}") for d in range(DT)]
            wq_t = [pers.tile([P, HDH], F16, tag=f"wq{d}", name=f"wq{d}") for d in range(DT)]
            wv_t = [pers.tile([P, HDH], F16, tag=f"wv{d}", name=f"wv{d}") for d in range(DT)]
            ones_mm = pers.tile([1, 512], F16, tag="ones_mm")
            ones_red = pers.tile([P, 64], F16, tag="ones_red")
            bq_sb = pers.tile([1, HDH], F16, tag="bq")
            bk_sb = pers.tile([1, HDH], F16, tag="bk")
            bv_sb = pers.tile([1, HDH], F16, tag="bv")
            dum_in = pers.tile([1, 16], F32, tag="dum_in")
            dum_out = pers.tile([1, 16], F16, tag="dum_out")

            # preload the exp table set while input DMAs run
            nc.vector.memset(dum_in, 0.0)
            nc.scalar.activation(out=dum_out, in_=dum_in, func=EXP)
            nc.vector.memset(ones_mm, 1.0)
            nc.vector.memset(ones_red, 1.0)
            nc.gpsimd.dma_start(out=bq_sb, in_=bq[None, :])
            nc.gpsimd.dma_start(out=bk_sb, in_=bk[None, :])
            nc.gpsimd.dma_start(out=bv_sb, in_=bv[None, :])

            # ---------------- input DMAs ----------------
            # weights on the gpsimd queue (small; gate the projections)
            for d in range(DT):
                nc.gpsimd.dma_start(out=wk_t[d], in_=wk[d * P : (d + 1) * P, :])
            for d in range(DT):
                nc.gpsimd.dma_start(out=wv_t[d], in_=wv[d * P : (d + 1) * P, :])
            for d in range(DT):
                nc.gpsimd.dma_start(out=wq_t[d], in_=wq[d * P : (d + 1) * P, :])
            for t in range(PAIRS):
                nc.gpsimd.dma_start(out=wo_sb[t], in_=wo[t * P : (t + 1) * P, :])

            # x loaded transposed via the M2S XBAR, split across both HWDGE
            # queues (sync + scalar)
            for d in range(DT):
                eng = nc.sync if d % 2 == 0 else nc.scalar
                eng.dma_start_transpose(out=xkv_t[d], in_=xkv[:, d * P : (d + 1) * P])
            for d in range(DT):
                eng = nc.sync if d % 2 == 0 else nc.scalar
                eng.dma_start_transpose(out=xq_t[d], in_=xq[:, d * P : (d + 1) * P])

            # ---------------- projection emitters ----------------
            def proj_chunk_T(x_tiles, w_tiles, b_sb, out_sb, t, c, d0, d1, ps=None):
                """Emit proj matmuls d0..d1 for chunk (t, c); finish + evacuate
                when d1 == DT. Returns the PSUM tile while the group is open."""
                if ps is None:
                    ps = pfil.tile([P, 512], F32, tag="fil", name="pjt")
                for d in range(d0, d1):
                    nc.tensor.matmul(
                        ps,
                        lhsT=w_tiles[d][:, t * P : (t + 1) * P],
                        rhs=x_tiles[d][:, c * 512 : (c + 1) * 512],
                        start=(d == 0),
                        stop=False,
                        skip_group_check=True,
                    )
                if d1 == DT:
                    nc.tensor.matmul(
                        ps,
                        lhsT=b_sb[:, t * P : (t + 1) * P],
                        rhs=ones_mm,
                        start=False,
                        stop=True,
                        skip_group_check=True,
                    )
                    nc.vector.tensor_copy(out=out_sb[:, c * 512 : (c + 1) * 512], in_=ps)
                    return None
                return ps

            def proj_chunk_v(st):
                """v_sb[st] = X[st] @ Wv + bv (natural layout)."""
                ps = pfil.tile([P, 512], F32, tag="fil", name="pjv")
                for d in range(DT):
                    nc.tensor.matmul(
                        ps,
                        lhsT=xkv_t[d][:, st * P : (st + 1) * P],
                        rhs=wv_t[d],
                        start=(d == 0),
                        stop=False,
                        skip_group_check=True,
                    )
                nc.tensor.matmul(
                    ps,
                    lhsT=ones_mm[:, :P],
                    rhs=bv_sb,
                    start=False,
                    stop=True,
                    skip_group_check=True,
                )
                nc.vector.tensor_copy(out=v_sb[st], in_=ps)

            # ---------------- prologue projections ----------------
            # k/q for pair 0 + all of V; pairs 1-3 stream in as fillers.
            for c in range(S // 512):
                proj_chunk_T(xkv_t, wk_t, bk_sb, kt_sb[0], 0, c, 0, DT)
            for c in range(S // 512):
                proj_chunk_T(xq_t, wq_t, bq_sb, qt_sb[0], 0, c, 0, DT)
            for st in range(NKT):
                proj_chunk_v(st)

            # ---------------- filler machinery ----------------
            # Each filler is ~1us of PE work; one is popped every other kt
            # window (PE slack per window is ~550ns).
            fillers = []

            def mk_proj_filler_halves(x_tiles, w_tiles, b_sb, out_sb, t, c):
                st = {}

                def f1():
                    st["ps"] = proj_chunk_T(
                        x_tiles, w_tiles, b_sb, out_sb, t, c, 0, DT // 2
                    )

                def f2():
                    proj_chunk_T(
                        x_tiles, w_tiles, b_sb, out_sb, t, c, DT // 2, DT,
                        ps=st["ps"],
                    )

                return [f1, f2]

            def mk_outproj_filler(qc, qt_, dc):
                def f():
                    po = pfil.tile([P, 512], F32, tag="fil", name="po")
                    for pr in range(PAIRS):
                        nc.tensor.matmul(
                            po,
                            lhsT=xts[pr][qc][:, qt_ * P : (qt_ + 1) * P],
                            rhs=wo_sb[pr][:, dc * 512 : (dc + 1) * 512],
                            start=(pr == 0),
                            stop=(pr == PAIRS - 1),
                            skip_group_check=True,
                        )
                    osb = osb_pool.tile([P, 512], F32, tag="osb", name="osb")
                    nc.vector.tensor_copy(out=osb, in_=po)
                    q0 = qc * QC + qt_ * P
                    nc.gpsimd.dma_start(
                        out=out[q0 : q0 + P, dc * 512 : (dc + 1) * 512], in_=osb
                    )
                return f

            proj_fillers = {
                pr: [
                    h
                    for c in range(S // 512)
                    for h in mk_proj_filler_halves(xkv_t, wk_t, bk_sb, kt_sb[pr], pr, c)
                ]
                + [
                    h
                    for c in range(S // 512)
                    for h in mk_proj_filler_halves(xq_t, wq_t, bq_sb, qt_sb[pr], pr, c)
                ]
                for pr in range(1, PAIRS)
            }

            # ---------------- attention stream ----------------
            xts = [[None] * NQC for _ in range(PAIRS)]  # xt_sb[pr][qc]

            def block_prologue(pr, qc):
                """Emit the first two scores-pair groups of block (pr, qc)."""
                ring = {}

                def emit_scores(kt):
                    ps = psc.tile([P, 2 * QC], F32, tag="sc", name="ps")
                    ksl = slice(kt * P, (kt + 1) * P)
                    qsl = slice(qc * QC, (qc + 1) * QC)
                    nc.tensor.matmul(
                        ps[:, 0:QC],
                        lhsT=kt_sb[pr][0:64, ksl],
                        rhs=qt_sb[pr][0:64, qsl],
                        start=True,
                        stop=True,
                        tile_position=(0, 0),
                    )
                    nc.tensor.matmul(
                        ps[:, QC : 2 * QC],
                        lhsT=kt_sb[pr][64:128, ksl],
                        rhs=qt_sb[pr][64:128, qsl],
                        start=True,
                        stop=True,
                        tile_position=(64, 0),
                    )
                    ring[kt] = ps

                emit_scores(0)
                emit_scores(1)
                return ring, emit_scores

            def block_body(pr, qc, ring, emit_scores):
                """The kt stream: exp, denominator accumulate, pv, fillers."""
                h0, h1 = 2 * pr, 2 * pr + 1
                acc = acc_pool.tile([P, 2 * QC], F16, tag="acc", name="acc")
                xt_ps = pxt.tile([P, QC], F32, tag="xt", name="xt")
                for kt in range(NKT):
                    ps = ring.pop(kt)
                    et = et_pool.tile([P, 2 * QC], F16, tag="et", name="et")
                    nc.scalar.activation(out=et, in_=ps, func=EXP, scale=0.125)
                    if kt == 0:
                        nc.vector.tensor_copy(out=acc, in_=et)
                    else:
                        nc.vector.tensor_add(out=acc, in0=acc, in1=et)
                    if kt + 2 < NKT:
                        emit_scores(kt + 2)
                    nc.tensor.matmul(
                        xt_ps[0:64, :],
                        lhsT=v_sb[kt][:, h0 * DH : (h0 + 1) * DH],
                        rhs=et[:, 0:QC],
                        start=(kt == 0),
                        stop=(kt == NKT - 1),
                        tile_position=(0, 0),
                        skip_group_check=True,
                    )
                    nc.tensor.matmul(
                        xt_ps[64:128, :],
                        lhsT=v_sb[kt][:, h1 * DH : (h1 + 1) * DH],
                        rhs=et[:, QC : 2 * QC],
                        start=(kt == 0),
                        stop=(kt == NKT - 1),
                        tile_position=(0, 64),
                        skip_group_check=True,
                    )
                    if kt % 2 == 1 and fillers:
                        fillers.pop(0)()
                return acc, xt_ps

            def block_epilogue(pr, qc, acc, xt_ps):
                """Denominator reduce+broadcast, reciprocal, normalize."""
                bs = pbs.tile([P, QC], F32, tag="bs", name="bs")
                nc.tensor.matmul(
                    bs[0:64, :],
                    lhsT=ones_red,
                    rhs=acc[:, 0:QC],
                    start=True,
                    stop=True,
                    tile_position=(0, 0),
                    skip_group_check=True,
                )
                nc.tensor.matmul(
                    bs[64:128, :],
                    lhsT=ones_red,
                    rhs=acc[:, QC : 2 * QC],
                    start=True,
                    stop=True,
                    tile_position=(0, 64),
                    skip_group_check=True,
                )
                rec = rec_pool.tile([P, QC], F32, tag="rec", name="rec")
                nc.vector.reciprocal_approx_fast(out=rec, in_=bs)
                xt_sb = xtsb_pool.tile([P, QC], F16, tag="xtsb", name="xtsb")
                nc.vector.tensor_mul(out=xt_sb, in0=xt_ps, in1=rec)
                xts[pr][qc] = xt_sb
                if pr == PAIRS - 1:
                    for qt_ in range(QC // P):
                        for dc in range(D // 512):
                            fillers.append(mk_outproj_filler(qc, qt_, dc))

            blocks = [(pr, qc) for pr in range(PAIRS) for qc in range(NQC)]
            pending = None  # (pr, qc, acc, xt_ps) of the previous block
            for pr, qc in blocks:
                if qc == 0 and pr + 1 < PAIRS:
                    fillers.extend(proj_fillers[pr + 1])
                ring, emit_scores = block_prologue(pr, qc)
                if pending is not None:
                    block_epilogue(*pending)
                acc, xt_ps = block_body(pr, qc, ring, emit_scores)
                pending = (pr, qc, acc, xt_ps)
            block_epilogue(*pending)

            # tail: remaining out-proj fillers of the last q-chunk
            while fillers:
                fillers.pop(0)()

    return nc


_NC_CACHE = None
LAST_RESULTS = None


def _get_nc():
    global _NC_CACHE
    if _NC_CACHE is None:
        nc = bacc.Bacc(None, target_bir_lowering=False)
        _emit(nc)
        nc.compile()
        _NC_CACHE = nc
    return _NC_CACHE


def kernel(**inputs):
    global LAST_RESULTS
    inputs_q = np.ascontiguousarray(inputs["inputs_q"], np.float16)
    inputs_kv = np.ascontiguousarray(inputs["inputs_kv"], np.float16)
    Wq = np.asarray(inputs["Wq"], np.float16)
    Wk = np.asarray(inputs["Wk"], np.float16)
    Wv = np.asarray(inputs["Wv"], np.float16)
    bq = np.asarray(inputs["bq"], np.float16)
    bk = np.asarray(inputs["bk"], np.float16)
    bv = np.asarray(inputs["bv"], np.float16)
    Wo = np.asarray(inputs["Wo"], np.float16)
    bo = np.asarray(inputs["bo"], np.float32)

    nc = _get_nc()

    in_maps = []
    for core in range(8):
        b, g = core // 2, core % 2
        hsl = slice(g * HC, (g + 1) * HC)
        in_maps.append(
            {
                "xq": inputs_q[b],
                "xkv": inputs_kv[b],
                "wq": np.ascontiguousarray(Wq[:, hsl, :].reshape(D, HDH)),
                "wk": np.ascontiguousarray(Wk[:, hsl, :].reshape(D, HDH)),
                "wv": np.ascontiguousarray(Wv[:, hsl, :].reshape(D, HDH)),
                "bq": np.ascontiguousarray(bq[hsl].reshape(HDH)),
                "bk": np.ascontiguousarray(bk[hsl].reshape(HDH)),
                "bv": np.ascontiguousarray(bv[hsl].reshape(HDH)),
                "wo": np.ascontiguousarray(Wo[hsl].reshape(HDH, D)),
            }
        )

    res = run_bass_kernel_spmd(
        nc,
        in_maps,
        core_ids=list(range(8)),
        trace=bool(int(os.environ.get("KERNEL_TRACE", "0"))),
    )
    LAST_RESULTS = res

    out = np.empty((B, S, D), np.float32)
    for b in range(B):
        out[b] = res.results[2 * b]["out"] + res.results[2 * b + 1]["out"] + bo
    return out


# revision 7
# speedup vs baseline: 1.1970x; 1.0061x over previous
"""Multi-head dot-product attention on 8 TRN2 NeuronCores.

Problem: B=4, S=2048, D=1024, H=16, DH=64 (fp32 reference).

Sharding: 8 shards = 4 batches x 2 head-halves. Each core computes, for one
batch b and 8 heads, the QKV projections, attention, and its partial output
projection. The host sums the two half-head partials per batch (the Wo
contraction all-reduce) and adds bo.

v2: the kernel is structured around the Scalar engine (ACT), which is the
critical resource: softmax needs exp of 8*2048*2048 = 33.5M elements per core
at 128 lanes @ 1.2 GHz ~= 255us when streamed back-to-back as [128,1024]
instructions. Everything else (PE matmuls ~240us, DVE ~210us) is scheduled to
hide underneath that stream:

  - attention is blocked as (head-pair pr, q-chunk qc=512, k-tile kt=128);
    per kt: one row-packed scores matmul pair (concurrent on the PE), ONE
    [128,1024] exp covering both heads, one DVE accumulate for the softmax
    denominator, and a col-packed PV matmul pair accumulating xT in PSUM.
  - the PE stream is software-pipelined: scores(kt+2) is emitted BEFORE
    pv(kt) so the exp stream never stalls behind the PE queue; the next
    block's first two scores are emitted before the current block's epilogue.
  - blocks are ordered pr-major; projections for pair pr+1 and the output
    projection run as "fillers" in the stream's PE slack (one ~1us filler
    per two kt windows).
  - PSUM budget (8 banks): scores 2x[128,1024]f32 (4) + xT [128,512]f32 (1)
    + denominator bs [128,512] (1) + 2 filler/out-proj slots (2).
"""

import os

import numpy as np

import concourse.bass as bass
from concourse import bacc
import concourse.mybir as mybir
import concourse.tile as tile
from concourse.bass_utils import run_bass_kernel_spmd

B, S, D, H, DH = 4, 2048, 1024, 16, 64
P = 128
HC = H // 2          # heads per core = 8
PAIRS = HC // 2      # head pairs per core = 4
DT = D // P          # projection contraction tiles = 8
NKT = S // P         # key tiles = 16
QC = 512             # q chunk (per attention block)
NQC = S // QC        # 4
HDH = HC * DH        # per-core Wo contraction = 512

F32 = mybir.dt.float32
F16 = mybir.dt.float16
EXP = mybir.ActivationFunctionType.Exp


def _emit(nc):
    xq = nc.dram_tensor("xq", [S, D], F16, kind="ExternalInput")
    xkv = nc.dram_tensor("xkv", [S, D], F16, kind="ExternalInput")
    wq = nc.dram_tensor("wq", [D, HDH], F16, kind="ExternalInput")
    wk = nc.dram_tensor("wk", [D, HDH], F16, kind="ExternalInput")
    wv = nc.dram_tensor("wv", [D, HDH], F16, kind="ExternalInput")
    bq = nc.dram_tensor("bq", [HDH], F16, kind="ExternalInput")
    bk = nc.dram_tensor("bk", [HDH], F16, kind="ExternalInput")
    bv = nc.dram_tensor("bv", [HDH], F16, kind="ExternalInput")
    wo = nc.dram_tensor("wo", [HDH, D], F16, kind="ExternalInput")
    out = nc.dram_tensor("out", [S, D], F32, kind="ExternalOutput")

    with tile.TileContext(nc) as tc:
        with (
            tc.tile_pool(name="persist", bufs=1) as pers,
            tc.tile_pool(name="etp", bufs=4) as et_pool,
            tc.tile_pool(name="accp", bufs=2) as acc_pool,
            tc.tile_pool(name="recp", bufs=2) as rec_pool,
            tc.tile_pool(name="xtsb", bufs=16) as xtsb_pool,
            tc.tile_pool(name="osbp", bufs=4) as osb_pool,
            tc.tile_pool(name="psc", bufs=2, space="PSUM") as psc,
            tc.tile_pool(name="pxt", bufs=1, space="PSUM") as pxt,
            tc.tile_pool(name="pbs", bufs=1, space="PSUM") as pbs,
            tc.tile_pool(name="pfil", bufs=2, space="PSUM") as pfil,
        ):
            # ---------------- persistent SBUF ----------------
            qt_sb = [pers.tile([P, S], F16, tag=f"qt{t}", name=f"qt{t}") for t in range(PAIRS)]
            kt_sb = [pers.tile([P, S], F16, tag=f"kt{t}", name=f"kt{t}") for t in range(PAIRS)]
            v_sb = [pers.tile([P, HDH], F16, tag=f"v{st}", name=f"v{st}") for st in range(NKT)]
            wo_sb = [pers.tile([P, D], F16, tag=f"wo{t}", name=f"wo{t}") for t in range(PAIRS)]
            xkv_t = [pers.tile([P, S], F16, tag=f"xkv{d}", name=f"xkv{d}") for d in range(DT)]
            xq_t = [pers.tile([P, S], F16, tag=f"xq{d}", name=f"xq{d}") for d in range(DT)]
            wk_t = [pers.tile([P, HDH], F16, tag=f"wk{d}", name=f"wk{d}") for d in range(DT)]
            wq_t = [pers.tile([P, HDH], F16, tag=f"wq{d}", name=f"wq{d}") for d in range(DT)]
            wv_t = [pers.tile([P, HDH], F16, tag=f"wv{d}", name=f"wv{d}") for d in range(DT)]
            ones_mm = pers.tile([1, 512], F16, tag="ones_mm")
            ones_red = pers.tile([P, 64], F16, tag="ones_red")
            bq_sb = pers.tile([1, HDH], F16, tag="bq")
            bk_sb = pers.tile([1, HDH], F16, tag="bk")
            bv_sb = pers.tile([1, HDH], F16, tag="bv")
            dum_in = pers.tile([1, 16], F32, tag="dum_in")
            dum_out = pers.tile([1, 16], F16, tag="dum_out")

            # preload the exp table set while input DMAs run
            nc.vector.memset(dum_in, 0.0)
            nc.scalar.activation(out=dum_out, in_=dum_in, func=EXP)
            nc.vector.memset(ones_mm, 1.0)
            nc.vector.memset(ones_red, 1.0)
            nc.gpsimd.dma_start(out=bq_sb, in_=bq[None, :])
            nc.gpsimd.dma_start(out=bk_sb, in_=bk[None, :])
            nc.gpsimd.dma_start(out=bv_sb, in_=bv[None, :])

            # ---------------- input DMAs ----------------
            # Weights first on the two HWDGE queues (they gate the
            # projections), then a pure run of XBAR transposes on the same
            # queues -- interleaving copy-mode DMAs with transposes thrashes
            # the XBAR mode and halves its throughput.
            for d in range(DT):
                nc.gpsimd.dma_start(out=wk_t[d], in_=wk[d * P : (d + 1) * P, :])
            for d in range(DT):
                nc.gpsimd.dma_start(out=wv_t[d], in_=wv[d * P : (d + 1) * P, :])
            for d in range(DT):
                nc.gpsimd.dma_start(out=wq_t[d], in_=wq[d * P : (d + 1) * P, :])
            for t in range(PAIRS):
                nc.gpsimd.dma_start(out=wo_sb[t], in_=wo[t * P : (t + 1) * P, :])

            # x loaded transposed via the M2S XBAR, split across both HWDGE
            # queues (sync + scalar)
            for d in range(DT):
                eng = nc.sync if d % 2 == 0 else nc.scalar
                eng.dma_start_transpose(out=xkv_t[d], in_=xkv[:, d * P : (d + 1) * P])
            for d in range(DT):
                eng = nc.sync if d % 2 == 0 else nc.scalar
                eng.dma_start_transpose(out=xq_t[d], in_=xq[:, d * P : (d + 1) * P])

            # ---------------- projection emitters ----------------
            def proj_chunk_T(x_tiles, w_tiles, b_sb, out_sb, t, c, d0, d1, ps=None):
                """Emit proj matmuls d0..d1 for chunk (t, c); finish + evacuate
                when d1 == DT. Returns the PSUM tile while the group is open."""
                if ps is None:
                    ps = pfil.tile([P, 512], F32, tag="fil", name="pjt")
                for d in range(d0, d1):
                    nc.tensor.matmul(
                        ps,
                        lhsT=w_tiles[d][:, t * P : (t + 1) * P],
                        rhs=x_tiles[d][:, c * 512 : (c + 1) * 512],
                        start=(d == 0),
                        stop=False,
                        skip_group_check=True,
                    )
                if d1 == DT:
                    nc.tensor.matmul(
                        ps,
                        lhsT=b_sb[:, t * P : (t + 1) * P],
                        rhs=ones_mm,
                        start=False,
                        stop=True,
                        skip_group_check=True,
                    )
                    nc.vector.tensor_copy(out=out_sb[:, c * 512 : (c + 1) * 512], in_=ps)
                    return None
                return ps

            def proj_chunk_v(st):
                """v_sb[st] = X[st] @ Wv + bv (natural layout)."""
                ps = pfil.tile([P, 512], F32, tag="fil", name="pjv")
                for d in range(DT):
                    nc.tensor.matmul(
                        ps,
                        lhsT=xkv_t[d][:, st * P : (st + 1) * P],
                        rhs=wv_t[d],
                        start=(d == 0),
                        stop=False,
                        skip_group_check=True,
                    )
                nc.tensor.matmul(
                    ps,
                    lhsT=ones_mm[:, :P],
                    rhs=bv_sb,
                    start=False,
                    stop=True,
                    skip_group_check=True,
                )
                nc.vector.tensor_copy(out=v_sb[st], in_=ps)

            # ---------------- prologue projections ----------------
            # k/q for pair 0 + all of V; pairs 1-3 stream in as fillers.
            for c in range(S // 512):
                proj_chunk_T(xkv_t, wk_t, bk_sb, kt_sb[0], 0, c, 0, DT)
            for c in range(S // 512):
                proj_chunk_T(xq_t, wq_t, bq_sb, qt_sb[0], 0, c, 0, DT)
            for st in range(NKT):
                proj_chunk_v(st)

            # ---------------- filler machinery ----------------
            # Each filler is ~1us of PE work; one is popped every other kt
            # window (PE slack per window is ~550ns).
            fillers = []

            def mk_proj_filler_halves(x_tiles, w_tiles, b_sb, out_sb, t, c):
                st = {}

                def f1():
                    st["ps"] = proj_chunk_T(
                        x_tiles, w_tiles, b_sb, out_sb, t, c, 0, DT // 2
                    )

                def f2():
                    proj_chunk_T(
                        x_tiles, w_tiles, b_sb, out_sb, t, c, DT // 2, DT,
                        ps=st["ps"],
                    )

                f1.heavy = f2.heavy = True
                return [f1, f2]

            def mk_outproj_filler(qc, qt_, dc):
                def f():
                    po = pfil.tile([P, 512], F32, tag="fil", name="po")
                    for pr in range(PAIRS):
                        nc.tensor.matmul(
                            po,
                            lhsT=xts[pr][qc][:, qt_ * P : (qt_ + 1) * P],
                            rhs=wo_sb[pr][:, dc * 512 : (dc + 1) * 512],
                            start=(pr == 0),
                            stop=(pr == PAIRS - 1),
                            skip_group_check=True,
                        )
                    osb = osb_pool.tile([P, 512], F32, tag="osb", name="osb")
                    nc.vector.tensor_copy(out=osb, in_=po)
                    q0 = qc * QC + qt_ * P
                    nc.gpsimd.dma_start(
                        out=out[q0 : q0 + P, dc * 512 : (dc + 1) * 512], in_=osb
                    )
                return f

            proj_fillers = {
                pr: [
                    h
                    for c in range(S // 512)
                    for h in mk_proj_filler_halves(xkv_t, wk_t, bk_sb, kt_sb[pr], pr, c)
                ]
                + [
                    h
                    for c in range(S // 512)
                    for h in mk_proj_filler_halves(xq_t, wq_t, bq_sb, qt_sb[pr], pr, c)
                ]
                for pr in range(1, PAIRS)
            }

            # ---------------- attention stream ----------------
            xts = [[None] * NQC for _ in range(PAIRS)]  # xt_sb[pr][qc]

            def block_prologue(pr, qc):
                """Emit the first two scores-pair groups of block (pr, qc)."""
                ring = {}

                def emit_scores(kt):
                    ps = psc.tile([P, 2 * QC], F32, tag="sc", name="ps")
                    ksl = slice(kt * P, (kt + 1) * P)
                    qsl = slice(qc * QC, (qc + 1) * QC)
                    nc.tensor.matmul(
                        ps[:, 0:QC],
                        lhsT=kt_sb[pr][0:64, ksl],
                        rhs=qt_sb[pr][0:64, qsl],
                        start=True,
                        stop=True,
                        tile_position=(0, 0),
                    )
                    nc.tensor.matmul(
                        ps[:, QC : 2 * QC],
                        lhsT=kt_sb[pr][64:128, ksl],
                        rhs=qt_sb[pr][64:128, qsl],
                        start=True,
                        stop=True,
                        tile_position=(64, 0),
                    )
                    ring[kt] = ps

                emit_scores(0)
                emit_scores(1)
                return ring, emit_scores

            def block_body(pr, qc, ring, emit_scores):
                """The kt stream: exp, denominator accumulate, pv, fillers."""
                h0, h1 = 2 * pr, 2 * pr + 1
                acc = acc_pool.tile([P, 2 * QC], F16, tag="acc", name="acc")
                xt_ps = pxt.tile([P, QC], F32, tag="xt", name="xt")
                for kt in range(NKT):
                    ps = ring.pop(kt)
                    et = et_pool.tile([P, 2 * QC], F16, tag="et", name="et")
                    nc.scalar.activation(out=et, in_=ps, func=EXP, scale=0.125)
                    if kt == 0:
                        nc.vector.tensor_copy(out=acc, in_=et)
                    else:
                        nc.vector.tensor_add(out=acc, in0=acc, in1=et)
                    if kt + 2 < NKT:
                        emit_scores(kt + 2)
                    nc.tensor.matmul(
                        xt_ps[0:64, :],
                        lhsT=v_sb[kt][:, h0 * DH : (h0 + 1) * DH],
                        rhs=et[:, 0:QC],
                        start=(kt == 0),
                        stop=(kt == NKT - 1),
                        tile_position=(0, 0),
                        skip_group_check=True,
                    )
                    nc.tensor.matmul(
                        xt_ps[64:128, :],
                        lhsT=v_sb[kt][:, h1 * DH : (h1 + 1) * DH],
                        rhs=et[:, QC : 2 * QC],
                        start=(kt == 0),
                        stop=(kt == NKT - 1),
                        tile_position=(0, 64),
                        skip_group_check=True,
                    )
                    if fillers:
                        # heavy (projection) fillers every 3rd window, light
                        # (out-proj) every other -- the PE slack per window is
                        # ~550ns vs ~1us / ~900ns of filler work.
                        heavy = getattr(fillers[0], "heavy", False)
                        if (kt % 3 == 1) if heavy else (kt % 2 == 1):
                            fillers.pop(0)()
                return acc, xt_ps

            def block_epilogue(pr, qc, acc, xt_ps):
                """Denominator reduce+broadcast, reciprocal, normalize."""
                bs = pbs.tile([P, QC], F32, tag="bs", name="bs")
                nc.tensor.matmul(
                    bs[0:64, :],
                    lhsT=ones_red,
                    rhs=acc[:, 0:QC],
                    start=True,
                    stop=True,
                    tile_position=(0, 0),
                    skip_group_check=True,
                )
                nc.tensor.matmul(
                    bs[64:128, :],
                    lhsT=ones_red,
                    rhs=acc[:, QC : 2 * QC],
                    start=True,
                    stop=True,
                    tile_position=(0, 64),
                    skip_group_check=True,
                )
                rec = rec_pool.tile([P, QC], F32, tag="rec", name="rec")
                nc.vector.reciprocal_approx_fast(out=rec, in_=bs)
                xt_sb = xtsb_pool.tile([P, QC], F16, tag="xtsb", name="xtsb")
                nc.vector.tensor_mul(out=xt_sb, in0=xt_ps, in1=rec)
                xts[pr][qc] = xt_sb
                if pr == PAIRS - 1:
                    for qt_ in range(QC // P):
                        for dc in range(D // 512):
                            fillers.append(mk_outproj_filler(qc, qt_, dc))

            blocks = [(pr, qc) for pr in range(PAIRS) for qc in range(NQC)]
            pending = None  # (pr, qc, acc, xt_ps) of the previous block
            for pr, qc in blocks:
                if qc == 0 and pr + 1 < PAIRS:
                    fillers.extend(proj_fillers[pr + 1])
                ring, emit_scores = block_prologue(pr, qc)
                if pending is not None:
                    block_epilogue(*pending)
                acc, xt_ps = block_body(pr, qc, ring, emit_scores)
                pending = (pr, qc, acc, xt_ps)
            block_epilogue(*pending)

            # tail: remaining out-proj fillers of the last q-chunk
            while fillers:
                fillers.pop(0)()

    return nc


_NC_CACHE = None
LAST_RESULTS = None


def _get_nc():
    global _NC_CACHE
    if _NC_CACHE is None:
        nc = bacc.Bacc(None, target_bir_lowering=False)
        _emit(nc)
        nc.compile()
        _NC_CACHE = nc
    return _NC_CACHE


def kernel(**inputs):
    global LAST_RESULTS
    inputs_q = np.ascontiguousarray(inputs["inputs_q"], np.float16)
    inputs_kv = np.ascontiguousarray(inputs["inputs_kv"], np.float16)
    Wq = np.asarray(inputs["Wq"], np.float16)
    Wk = np.asarray(inputs["Wk"], np.float16)
    Wv = np.asarray(inputs["Wv"], np.float16)
    bq = np.asarray(inputs["bq"], np.float16)
    bk = np.asarray(inputs["bk"], np.float16)
    bv = np.asarray(inputs["bv"], np.float16)
    Wo = np.asarray(inputs["Wo"], np.float16)
    bo = np.asarray(inputs["bo"], np.float32)

    nc = _get_nc()

    in_maps = []
    for core in range(8):
        b, g = core // 2, core % 2
        hsl = slice(g * HC, (g + 1) * HC)
        in_maps.append(
            {
                "xq": inputs_q[b],
                "xkv": inputs_kv[b],
                "wq": np.ascontiguousarray(Wq[:, hsl, :].reshape(D, HDH)),
                "wk": np.ascontiguousarray(Wk[:, hsl, :].reshape(D, HDH)),
                "wv": np.ascontiguousarray(Wv[:, hsl, :].reshape(D, HDH)),
                "bq": np.ascontiguousarray(bq[hsl].reshape(HDH)),
                "bk": np.ascontiguousarray(bk[hsl].reshape(HDH)),
                "bv": np.ascontiguousarray(bv[hsl].reshape(HDH)),
                "wo": np.ascontiguousarray(Wo[hsl].reshape(HDH, D)),
            }
        )

    res = run_bass_kernel_spmd(
        nc,
        in_maps,
        core_ids=list(range(8)),
        trace=bool(int(os.environ.get("KERNEL_TRACE", "0"))),
    )
    LAST_RESULTS = res

    out = np.empty((B, S, D), np.float32)
    for b in range(B):
        out[b] = res.results[2 * b]["out"] + res.results[2 * b + 1]["out"] + bo
    return out


# revision 8
# speedup vs baseline: 1.2137x; 1.0140x over previous
"""Multi-head dot-product attention on 8 TRN2 NeuronCores.

Problem: B=4, S=2048, D=1024, H=16, DH=64 (fp32 reference).

Sharding: 8 shards = 4 batches x 2 head-halves. Each core computes, for one
batch b and 8 heads, the QKV projections, attention, and its partial output
projection. The host sums the two half-head partials per batch (the Wo
contraction all-reduce) and adds bo.

v2: the kernel is structured around the Scalar engine (ACT), which is the
critical resource: softmax needs exp of 8*2048*2048 = 33.5M elements per core
at 128 lanes @ 1.2 GHz ~= 255us when streamed back-to-back as [128,1024]
instructions. Everything else (PE matmuls ~240us, DVE ~210us) is scheduled to
hide underneath that stream:

  - attention is blocked as (head-pair pr, q-chunk qc=512, k-tile kt=128);
    per kt: one row-packed scores matmul pair (concurrent on the PE), ONE
    [128,1024] exp covering both heads, one DVE accumulate for the softmax
    denominator, and a col-packed PV matmul pair accumulating xT in PSUM.
  - the PE stream is software-pipelined: scores(kt+2) is emitted BEFORE
    pv(kt) so the exp stream never stalls behind the PE queue; the next
    block's first two scores are emitted before the current block's epilogue.
  - blocks are ordered pr-major; projections for pair pr+1 and the output
    projection run as "fillers" in the stream's PE slack (one ~1us filler
    per two kt windows).
  - PSUM budget (8 banks): scores 2x[128,1024]f32 (4) + xT [128,512]f32 (1)
    + denominator bs [128,512] (1) + 2 filler/out-proj slots (2).
"""

import os

import numpy as np

import concourse.bass as bass
from concourse import bacc
import concourse.mybir as mybir
import concourse.tile as tile
from concourse.bass_utils import run_bass_kernel_spmd

B, S, D, H, DH = 4, 2048, 1024, 16, 64
P = 128
HC = H // 2          # heads per core = 8
PAIRS = HC // 2      # head pairs per core = 4
DT = D // P          # projection contraction tiles = 8
NKT = S // P         # key tiles = 16
QC = 512             # q chunk (per attention block)
NQC = S // QC        # 4
HDH = HC * DH        # per-core Wo contraction = 512

F32 = mybir.dt.float32
F16 = mybir.dt.float16
EXP = mybir.ActivationFunctionType.Exp


def _emit(nc):
    xq = nc.dram_tensor("xq", [S, D], F16, kind="ExternalInput")
    xkv = nc.dram_tensor("xkv", [S, D], F16, kind="ExternalInput")
    wq = nc.dram_tensor("wq", [D, HDH], F16, kind="ExternalInput")
    wk = nc.dram_tensor("wk", [D, HDH], F16, kind="ExternalInput")
    wv = nc.dram_tensor("wv", [D, HDH], F16, kind="ExternalInput")
    bq = nc.dram_tensor("bq", [HDH], F16, kind="ExternalInput")
    bk = nc.dram_tensor("bk", [HDH], F16, kind="ExternalInput")
    bv = nc.dram_tensor("bv", [HDH], F16, kind="ExternalInput")
    wo = nc.dram_tensor("wo", [HDH, D], F16, kind="ExternalInput")
    out = nc.dram_tensor("out", [S, D], F32, kind="ExternalOutput")

    with tile.TileContext(nc) as tc:
        with (
            tc.tile_pool(name="persist", bufs=1) as pers,
            tc.tile_pool(name="etp", bufs=4) as et_pool,
            tc.tile_pool(name="accp", bufs=2) as acc_pool,
            tc.tile_pool(name="recp", bufs=2) as rec_pool,
            tc.tile_pool(name="xtsb", bufs=16) as xtsb_pool,
            tc.tile_pool(name="osbp", bufs=4) as osb_pool,
            tc.tile_pool(name="psc", bufs=2, space="PSUM") as psc,
            tc.tile_pool(name="pxt", bufs=1, space="PSUM") as pxt,
            tc.tile_pool(name="pbs", bufs=1, space="PSUM") as pbs,
            tc.tile_pool(name="pfil", bufs=2, space="PSUM") as pfil,
        ):
            # ---------------- persistent SBUF ----------------
            qt_sb = [pers.tile([P, S], F16, tag=f"qt{t}", name=f"qt{t}") for t in range(PAIRS)]
            kt_sb = [pers.tile([P, S], F16, tag=f"kt{t}", name=f"kt{t}") for t in range(PAIRS)]
            v_sb = [pers.tile([P, HDH], F16, tag=f"v{st}", name=f"v{st}") for st in range(NKT)]
            wo_sb = [pers.tile([P, D], F16, tag=f"wo{t}", name=f"wo{t}") for t in range(PAIRS)]
            xkv_t = [pers.tile([P, S], F16, tag=f"xkv{d}", name=f"xkv{d}") for d in range(DT)]
            xq_t = [pers.tile([P, S], F16, tag=f"xq{d}", name=f"xq{d}") for d in range(DT)]
            wk_t = [pers.tile([P, HDH], F16, tag=f"wk{d}", name=f"wk{d}") for d in range(DT)]
            wq_t = [pers.tile([P, HDH], F16, tag=f"wq{d}", name=f"wq{d}") for d in range(DT)]
            wv_t = [pers.tile([P, HDH], F16, tag=f"wv{d}", name=f"wv{d}") for d in range(DT)]
            ones_mm = pers.tile([1, 512], F16, tag="ones_mm")
            ones_red = pers.tile([P, 64], F16, tag="ones_red")
            bq_sb = pers.tile([1, HDH], F16, tag="bq")
            bk_sb = pers.tile([1, HDH], F16, tag="bk")
            bv_sb = pers.tile([1, HDH], F16, tag="bv")
            dum_in = pers.tile([1, 16], F32, tag="dum_in")
            dum_out = pers.tile([1, 16], F16, tag="dum_out")

            # preload the exp table set while input DMAs run
            nc.vector.memset(dum_in, 0.0)
            nc.scalar.activation(out=dum_out, in_=dum_in, func=EXP)
            nc.vector.memset(ones_mm, 1.0)
            nc.vector.memset(ones_red, 1.0)
            nc.gpsimd.dma_start(out=bq_sb, in_=bq[None, :])
            nc.gpsimd.dma_start(out=bk_sb, in_=bk[None, :])
            nc.gpsimd.dma_start(out=bv_sb, in_=bv[None, :])

            # ---------------- input DMAs ----------------
            # Weights first on the two HWDGE queues (they gate the
            # projections), then a pure run of XBAR transposes on the same
            # queues -- interleaving copy-mode DMAs with transposes thrashes
            # the XBAR mode and halves its throughput.
            w_insts = []
            for d in range(DT):
                w_insts.append(nc.gpsimd.dma_start(out=wk_t[d], in_=wk[d * P : (d + 1) * P, :]))
            for d in range(DT):
                w_insts.append(nc.gpsimd.dma_start(out=wv_t[d], in_=wv[d * P : (d + 1) * P, :]))
            for d in range(DT):
                w_insts.append(nc.gpsimd.dma_start(out=wq_t[d], in_=wq[d * P : (d + 1) * P, :]))
            for t in range(PAIRS):
                w_insts.append(nc.gpsimd.dma_start(out=wo_sb[t], in_=wo[t * P : (t + 1) * P, :]))

            # x loaded transposed via the M2S XBAR, split across both HWDGE
            # queues (sync + scalar). Delay the transposes until the weight
            # copy-DMAs have drained so the XBAR runs a clean transpose-only
            # burst (concurrent copy traffic serializes it).
            first_t = {}
            for d in range(DT):
                eng = nc.sync if d % 2 == 0 else nc.scalar
                ti = eng.dma_start_transpose(out=xkv_t[d], in_=xkv[:, d * P : (d + 1) * P])
                if d < 2:
                    first_t[d] = ti
            for d in range(DT):
                eng = nc.sync if d % 2 == 0 else nc.scalar
                eng.dma_start_transpose(out=xq_t[d], in_=xq[:, d * P : (d + 1) * P])
            for ti in first_t.values():
                tile.add_dep_helper(
                    w_insts[-1].ins, ti.ins, sync=True, reason="xbar after weights"
                )

            # ---------------- projection emitters ----------------
            def proj_chunk_T(x_tiles, w_tiles, b_sb, out_sb, t, c, d0, d1, ps=None):
                """Emit proj matmuls d0..d1 for chunk (t, c); finish + evacuate
                when d1 == DT. Returns the PSUM tile while the group is open."""
                if ps is None:
                    ps = pfil.tile([P, 512], F32, tag="fil", name="pjt")
                for d in range(d0, d1):
                    nc.tensor.matmul(
                        ps,
                        lhsT=w_tiles[d][:, t * P : (t + 1) * P],
                        rhs=x_tiles[d][:, c * 512 : (c + 1) * 512],
                        start=(d == 0),
                        stop=False,
                        skip_group_check=True,
                    )
                if d1 == DT:
                    nc.tensor.matmul(
                        ps,
                        lhsT=b_sb[:, t * P : (t + 1) * P],
                        rhs=ones_mm,
                        start=False,
                        stop=True,
                        skip_group_check=True,
                    )
                    nc.vector.tensor_copy(out=out_sb[:, c * 512 : (c + 1) * 512], in_=ps)
                    return None
                return ps

            def proj_chunk_v(st):
                """v_sb[st] = X[st] @ Wv + bv (natural layout)."""
                ps = pfil.tile([P, 512], F32, tag="fil", name="pjv")
                for d in range(DT):
                    nc.tensor.matmul(
                        ps,
                        lhsT=xkv_t[d][:, st * P : (st + 1) * P],
                        rhs=wv_t[d],
                        start=(d == 0),
                        stop=False,
                        skip_group_check=True,
                    )
                nc.tensor.matmul(
                    ps,
                    lhsT=ones_mm[:, :P],
                    rhs=bv_sb,
                    start=False,
                    stop=True,
                    skip_group_check=True,
                )
                nc.vector.tensor_copy(out=v_sb[st], in_=ps)

            # ---------------- prologue projections ----------------
            # k/q for pair 0 + all of V; pairs 1-3 stream in as fillers.
            for c in range(S // 512):
                proj_chunk_T(xkv_t, wk_t, bk_sb, kt_sb[0], 0, c, 0, DT)
            for c in range(S // 512):
                proj_chunk_T(xq_t, wq_t, bq_sb, qt_sb[0], 0, c, 0, DT)
            for st in range(NKT):
                proj_chunk_v(st)

            # ---------------- filler machinery ----------------
            # Each filler is ~1us of PE work; one is popped every other kt
            # window (PE slack per window is ~550ns).
            fillers = []

            def mk_proj_filler_halves(x_tiles, w_tiles, b_sb, out_sb, t, c):
                st = {}

                def f1():
                    st["ps"] = proj_chunk_T(
                        x_tiles, w_tiles, b_sb, out_sb, t, c, 0, DT // 2
                    )

                def f2():
                    proj_chunk_T(
                        x_tiles, w_tiles, b_sb, out_sb, t, c, DT // 2, DT,
                        ps=st["ps"],
                    )

                f1.heavy = f2.heavy = True
                return [f1, f2]

            def mk_outproj_filler(qc, qt_, dc):
                def f():
                    po = pfil.tile([P, 512], F32, tag="fil", name="po")
                    for pr in range(PAIRS):
                        nc.tensor.matmul(
                            po,
                            lhsT=xts[pr][qc][:, qt_ * P : (qt_ + 1) * P],
                            rhs=wo_sb[pr][:, dc * 512 : (dc + 1) * 512],
                            start=(pr == 0),
                            stop=(pr == PAIRS - 1),
                            skip_group_check=True,
                        )
                    osb = osb_pool.tile([P, 512], F32, tag="osb", name="osb")
                    nc.vector.tensor_copy(out=osb, in_=po)
                    q0 = qc * QC + qt_ * P
                    nc.gpsimd.dma_start(
                        out=out[q0 : q0 + P, dc * 512 : (dc + 1) * 512], in_=osb
                    )
                return f

            proj_fillers = {
                pr: [
                    h
                    for c in range(S // 512)
                    for h in mk_proj_filler_halves(xkv_t, wk_t, bk_sb, kt_sb[pr], pr, c)
                ]
                + [
                    h
                    for c in range(S // 512)
                    for h in mk_proj_filler_halves(xq_t, wq_t, bq_sb, qt_sb[pr], pr, c)
                ]
                for pr in range(1, PAIRS)
            }

            # ---------------- attention stream ----------------
            xts = [[None] * NQC for _ in range(PAIRS)]  # xt_sb[pr][qc]

            def block_prologue(pr, qc):
                """Emit the first two scores-pair groups of block (pr, qc)."""
                ring = {}

                def emit_scores(kt):
                    ps = psc.tile([P, 2 * QC], F32, tag="sc", name="ps")
                    ksl = slice(kt * P, (kt + 1) * P)
                    qsl = slice(qc * QC, (qc + 1) * QC)
                    nc.tensor.matmul(
                        ps[:, 0:QC],
                        lhsT=kt_sb[pr][0:64, ksl],
                        rhs=qt_sb[pr][0:64, qsl],
                        start=True,
                        stop=True,
                        tile_position=(0, 0),
                    )
                    nc.tensor.matmul(
                        ps[:, QC : 2 * QC],
                        lhsT=kt_sb[pr][64:128, ksl],
                        rhs=qt_sb[pr][64:128, qsl],
                        start=True,
                        stop=True,
                        tile_position=(64, 0),
                    )
                    ring[kt] = ps

                emit_scores(0)
                emit_scores(1)
                return ring, emit_scores

            def block_body(pr, qc, ring, emit_scores):
                """The kt stream: exp, denominator accumulate, pv, fillers."""
                h0, h1 = 2 * pr, 2 * pr + 1
                acc = acc_pool.tile([P, 2 * QC], F16, tag="acc", name="acc")
                xt_ps = pxt.tile([P, QC], F32, tag="xt", name="xt")
                for kt in range(NKT):
                    ps = ring.pop(kt)
                    et = et_pool.tile([P, 2 * QC], F16, tag="et", name="et")
                    nc.scalar.activation(out=et, in_=ps, func=EXP, scale=0.125)
                    if kt == 0:
                        nc.vector.tensor_copy(out=acc, in_=et)
                    else:
                        nc.vector.tensor_add(out=acc, in0=acc, in1=et)
                    if kt + 2 < NKT:
                        emit_scores(kt + 2)
                    nc.tensor.matmul(
                        xt_ps[0:64, :],
                        lhsT=v_sb[kt][:, h0 * DH : (h0 + 1) * DH],
                        rhs=et[:, 0:QC],
                        start=(kt == 0),
                        stop=(kt == NKT - 1),
                        tile_position=(0, 0),
                        skip_group_check=True,
                    )
                    nc.tensor.matmul(
                        xt_ps[64:128, :],
                        lhsT=v_sb[kt][:, h1 * DH : (h1 + 1) * DH],
                        rhs=et[:, QC : 2 * QC],
                        start=(kt == 0),
                        stop=(kt == NKT - 1),
                        tile_position=(0, 64),
                        skip_group_check=True,
                    )
                    if fillers:
                        # heavy (projection) fillers every 3rd window, light
                        # (out-proj) every other -- the PE slack per window is
                        # ~550ns vs ~1us / ~900ns of filler work.
                        heavy = getattr(fillers[0], "heavy", False)
                        if (kt % 3 == 1) if heavy else (kt % 2 == 1):
                            fillers.pop(0)()
                return acc, xt_ps

            def block_epilogue(pr, qc, acc, xt_ps):
                """Denominator reduce+broadcast, reciprocal, normalize."""
                bs = pbs.tile([P, QC], F32, tag="bs", name="bs")
                nc.tensor.matmul(
                    bs[0:64, :],
                    lhsT=ones_red,
                    rhs=acc[:, 0:QC],
                    start=True,
                    stop=True,
                    tile_position=(0, 0),
                    skip_group_check=True,
                )
                nc.tensor.matmul(
                    bs[64:128, :],
                    lhsT=ones_red,
                    rhs=acc[:, QC : 2 * QC],
                    start=True,
                    stop=True,
                    tile_position=(0, 64),
                    skip_group_check=True,
                )
                rec = rec_pool.tile([P, QC], F32, tag="rec", name="rec")
                nc.vector.reciprocal_approx_fast(out=rec, in_=bs)
                xt_sb = xtsb_pool.tile([P, QC], F16, tag="xtsb", name="xtsb")
                nc.vector.tensor_mul(out=xt_sb, in0=xt_ps, in1=rec)
                xts[pr][qc] = xt_sb
                if pr == PAIRS - 1:
                    for qt_ in range(QC // P):
                        for dc in range(D // 512):
                            fillers.append(mk_outproj_filler(qc, qt_, dc))

            blocks = [(pr, qc) for pr in range(PAIRS) for qc in range(NQC)]
            pending = None  # (pr, qc, acc, xt_ps) of the previous block
            for pr, qc in blocks:
                if qc == 0 and pr + 1 < PAIRS:
                    fillers.extend(proj_fillers[pr + 1])
                ring, emit_scores = block_prologue(pr, qc)
                if pending is not None:
                    block_epilogue(*pending)
                acc, xt_ps = block_body(pr, qc, ring, emit_scores)
                pending = (pr, qc, acc, xt_ps)
            block_epilogue(*pending)

            # tail: remaining out-proj fillers of the last q-chunk
            while fillers:
                fillers.pop(0)()

    return nc


_NC_CACHE = None
LAST_RESULTS = None


def _get_nc():
    global _NC_CACHE
    if _NC_CACHE is None:
        nc = bacc.Bacc(None, target_bir_lowering=False)
        _emit(nc)
        nc.compile()
        _NC_CACHE = nc
    return _NC_CACHE


def kernel(**inputs):
    global LAST_RESULTS
    inputs_q = np.ascontiguousarray(inputs["inputs_q"], np.float16)
    inputs_kv = np.ascontiguousarray(inputs["inputs_kv"], np.float16)
    Wq = np.asarray(inputs["Wq"], np.float16)
    Wk = np.asarray(inputs["Wk"], np.float16)
    Wv = np.asarray(inputs["Wv"], np.float16)
    bq = np.asarray(inputs["bq"], np.float16)
    bk = np.asarray(inputs["bk"], np.float16)
    bv = np.asarray(inputs["bv"], np.float16)
    Wo = np.asarray(inputs["Wo"], np.float16)
    bo = np.asarray(inputs["bo"], np.float32)

    nc = _get_nc()

    in_maps = []
    for core in range(8):
        b, g = core // 2, core % 2
        hsl = slice(g * HC, (g + 1) * HC)
        in_maps.append(
            {
                "xq": inputs_q[b],
                "xkv": inputs_kv[b],
                "wq": np.ascontiguousarray(Wq[:, hsl, :].reshape(D, HDH)),
                "wk": np.ascontiguousarray(Wk[:, hsl, :].reshape(D, HDH)),
                "wv": np.ascontiguousarray(Wv[:, hsl, :].reshape(D, HDH)),
                "bq": np.ascontiguousarray(bq[hsl].reshape(HDH)),
                "bk": np.ascontiguousarray(bk[hsl].reshape(HDH)),
                "bv": np.ascontiguousarray(bv[hsl].reshape(HDH)),
                "wo": np.ascontiguousarray(Wo[hsl].reshape(HDH, D)),
            }
        )

    res = run_bass_kernel_spmd(
        nc,
        in_maps,
        core_ids=list(range(8)),
        trace=bool(int(os.environ.get("KERNEL_TRACE", "0"))),
    )
    LAST_RESULTS = res

    out = np.empty((B, S, D), np.float32)
    for b in range(B):
        out[b] = res.results[2 * b]["out"] + res.results[2 * b + 1]["out"] + bo
    return out
